# revision 17
# baseline (speedup 1.0000x reference)
"""Causal self-attention (B=2, T=2048, C=1024, NH=16, HD=64) on 8 TRN2 cores.

Sharding: TP over heads x DP over batch. Core i handles batch i//4 and
heads [4*(i%4) .. 4*(i%4)+4). v4 design:

  1. Progressive fat input DMA: x^T lands as 8 half-quarter descriptors
     split across the sync+scalar HWDGE queues (c 0-3 / c 4-7), weights
     on the gpsimd SWDGE ordered by first use (w_qk m01 before w_v
     before w_qk m23), so the first QK matmul issues at ~10us and is
     paced by arrival, never by a monolithic transfer.
  2. QKV projection pipelined per t-quarter (QK m-tiles + V quad); PSUM
     evictions on DVE (per-partition-bias tensor_scalar_add / strided
     copy); ACT does exp only. Full-contraction warmup matmuls ramp the
     PE p-state during boot.
  3. Flash-style causal attention per head-pair in S^T=[k,q] layout,
     chunk order 0,1,2,3. One ScalarE Exp per ki covers both heads;
     causal mask on the diagonal block only (DVE); denominators
     reciprocal'd on DVE, broadcast via GpSimd partition_broadcast.
  4. Output stage: per-PAIR AllGather (128 rows, 8 collectives) issued
     right behind each pair's y^T store; projection for chunk qc runs
     one chunk later (yt loads via gpsimd so no compute queue ever
     FIFO-blocks on a collective). Only AG(3,p1)+proj(3) are exposed.

Compute dtype bf16 (fp32 PSUM accumulation everywhere).
"""
import sys
import types

import numpy as np
import ml_dtypes

import concourse.bass as bass
import concourse.bacc as bacc
import concourse.tile as tile
import concourse.mybir as mybir
from concourse.bass_utils import run_bass_kernel_spmd

B, T, C, NH, HD = 2, 2048, 1024, 16, 64
N_CORES = 8
TP, DP = 4, 2
HLOC = NH // TP            # 4 heads per core
DLOC = HLOC * HD           # 256
GROUPS = [[0, 1, 2, 3], [4, 5, 6, 7]]
NKT = T // 128             # 16 k-tiles / t-tiles
NCT = C // 128             # 8 c_in tiles
ES = 512

F32 = mybir.dt.float32
F16 = mybir.dt.float16
BF16 = mybir.dt.bfloat16
AF = mybir.ActivationFunctionType
BF16_NP = ml_dtypes.bfloat16

_CACHED_NC = None
N_WARMUP = 16


def _install_ntff_hook():
    """Register the axon NTFF profiling shim if the image lacks it."""
    if "antenv.axon_hooks" in sys.modules:
        return
    try:
        from trn_agent_boot.trn_boot import _ntff_profile_via_ctypes
        hook = _ntff_profile_via_ctypes("/opt/axon/libaxon_pjrt.so")
        import antenv
        mod = types.ModuleType("antenv.axon_hooks")
        mod.get_axon_ntff_profile_hook = lambda: hook
        mod.set_axon_ntff_profile_hook = lambda h: None
        sys.modules["antenv.axon_hooks"] = mod
        antenv.axon_hooks = mod
    except Exception:
        pass


def build_kernel_body(nc, tc, es, d):
    sbuf = es.enter_context(tc.tile_pool(name="sbuf", bufs=1))
    sbuf2 = es.enter_context(tc.tile_pool(name="sbuf2", bufs=2))
    ppool = es.enter_context(tc.tile_pool(name="ppool", bufs=4))
    spool = es.enter_context(tc.tile_pool(name="spool", bufs=2, space="PSUM"))
    ypool = es.enter_context(tc.tile_pool(name="ypool", bufs=4, space="PSUM"))

    # ---- constants ---------------------------------------------------------
    ones1 = sbuf.tile([1, 128], BF16, tag="ones1")
    nc.vector.memset(ones1[:], 1.0)
    warma = sbuf.tile([128, 128], BF16, tag="warma")
    nc.vector.memset(warma[:], 0.01)
    warmb = sbuf.tile([128, 512], BF16, tag="warmb")
    nc.vector.memset(warmb[:], 0.01)
    dummy = sbuf.tile([1, 1], F32, tag="dummy")
    nc.vector.memset(dummy[:], 0.0)
    # trigger the exp table load during boot, before the first real exp
    dummy2 = sbuf.tile([1, 1], BF16, tag="dummy2")
    nc.scalar.activation(dummy2[:], dummy[:], AF.Exp, scale=1.0)

    # ---- input DMAs --------------------------------------------------------
    # All inputs are HOST-pre-tiled into [128, ...] partition-major layouts so
    # every descriptor is a plain 2D copy with >=2KB contiguous runs.
    # gpsimd SWDGE: weights + small consts, ordered by first use.
    mask = sbuf.tile([128, 128], BF16, tag="mask")
    bqk = sbuf.tile([128, 4], F32, tag="bqk")        # col m = bias of m-tile
    bv = sbuf.tile([1, DLOC], BF16, tag="bv")
    bpt = sbuf.tile([128, 2], F32, tag="bp")
    # wqk[128, half, c, 256]: half 0 = m-tiles 0,1 (q01|k01), half 1 = m-tiles
    # 2,3 (q23|k23); block c holds w_qk rows [128c, 128c+128). lhsT for
    # (m, c) = wqk[:, m//2, c, (m%2)*128 : +128].
    wqk = sbuf.tile([128, 2, NCT, 256], BF16, tag="wqk")
    wv = sbuf.tile([128, NCT, DLOC], BF16, tag="wv")
    wp = sbuf.tile([128, NCT, DLOC], BF16, tag="wp")
    nc.sync.dma_start(wqk[:, 0], d["w_qk"][:, 0:NCT * 256])
    nc.scalar.dma_start(wqk[:, 1], d["w_qk"][:, NCT * 256:NCT * 512])
    nc.gpsimd.dma_start(bqk[:], d["b_qk"][:])
    nc.gpsimd.dma_start(mask[:], d["mask"][:])
    nc.gpsimd.dma_start(bv[:], d["b_v"][:])
    nc.gpsimd.dma_start(wv[:], d["w_v"][:])
    nc.gpsimd.dma_start(bpt[:], d["b_p"][:])
    nc.gpsimd.dma_start(wp[:], d["w_proj"][:])

    # x^T quarters: xq[tq][128, c, 512]; host layout [128, (tq c m)].
    # Quarter 0 split in c-pairs across sync+scalar so QK starts earliest.
    xq = [sbuf.tile([128, NCT, 512], BF16, tag=f"xq{t}", name=f"xq{t}")
          for t in range(4)]
    for c in range(NCT):
        eng = nc.sync if c % 2 == 0 else nc.scalar
        eng.dma_start(xq[0][:, c:c + 1, :],
                      d["xT"][:, 512 * c:512 * c + 512])
    for tq in range(1, 4):
        base = tq * 4096
        nc.sync.dma_start(xq[tq][:, 0:4, :],
                          d["xT"][:, base:base + 2048])
        nc.scalar.dma_start(xq[tq][:, 4:8, :],
                            d["xT"][:, base + 2048:base + 4096])

    # ---- warmup collective: pay the ncfw cold-start during the input DMA ---
    nc.gpsimd.collective_compute(
        "AllGather",
        mybir.AluOpType.bypass,
        replica_groups=GROUPS,
        ins=[d["warmag"][0:1, :].opt()],
        outs=[d["warmag"][1:5, :].opt()],
    )

    # ---- PE warmup during input DMA (full-K so the HAM actually ramps) -----
    wps = spool.tile([128, 1024], F32, tag="S", name="warm")
    for i in range(N_WARMUP):
        nc.tensor.matmul(wps[:, 0:512], warma[:], warmb[:],
                         start=True, stop=True)

    # ---- QKV projection, pipelined per t-quarter ---------------------------
    # qkT[m]: [128, T] bf16; m=0: Q^T heads 0,1  m=1: K^T heads 0,1
    #                        m=2: Q^T heads 2,3  m=3: K^T heads 2,3
    qkT = [sbuf.tile([128, T], BF16, tag=f"qkT{m}", name=f"qkT{m}")
           for m in range(4)]
    vsb = [sbuf.tile([128, HLOC * (HD + 1)], BF16, tag=f"vsb{tt}",
                     name=f"vsb{tt}") for tt in range(NKT)]

    def emit_qk_quarter(tq):
        off = tq * 512
        for mp in range(2):            # psum tile holds m=2mp, 2mp+1
            ps = spool.tile([128, 1024], F32, tag="S", name=f"qkps{tq}{mp}")
            for m in (2 * mp, 2 * mp + 1):
                for c in range(NCT):
                    nc.tensor.matmul(
                        ps[:, (m % 2) * 512:(m % 2) * 512 + 512],
                        wqk[:, m // 2, c, (m % 2) * 128:(m % 2) * 128 + 128],
                        xq[tq][:, c, :],
                        start=(c == 0), stop=(c == NCT - 1))
            for m in (2 * mp, 2 * mp + 1):
                nc.vector.tensor_scalar_add(
                    qkT[m][:, off:off + 512],
                    ps[:, (m % 2) * 512:(m % 2) * 512 + 512],
                    bqk[:, m:m + 1])

    def emit_v_quarter(tq):
        ps = spool.tile([128, 1024], F32, tag="S", name=f"vps{tq}")
        for j in range(4):
            for c in range(NCT):
                nc.tensor.matmul(
                    ps[:, j * 256:(j + 1) * 256],
                    xq[tq][:, c, j * 128:(j + 1) * 128], wv[:, c, :],
                    start=(c == 0), stop=False)
            nc.tensor.matmul(ps[:, j * 256:(j + 1) * 256], ones1[:], bv[:],
                             start=False, stop=True)
        for j in range(4):
            tt = 4 * tq + j
            vgrp = vsb[tt][:].rearrange("p (h x) -> p h x", h=HLOC)
            nc.vector.tensor_copy(
                vgrp[:, :, 0:HD],
                ps[:, j * 256:(j + 1) * 256].rearrange(
                    "p (h x) -> p h x", h=HLOC))
            nc.vector.memset(vgrp[:, :, HD:HD + 1], 1.0)

    # ---- attention inner loop ---------------------------------------------
    yn = [sbuf.tile([128, T], BF16, tag=f"yn{p}", name=f"yn{p}")
          for p in range(2)]

    def emit_att(qc, p):
        qt, kt = qkT[2 * p], qkT[2 * p + 1]
        nki = 4 * qc + 4
        yps = [ypool.tile([65, 512], F32, tag="y", name=f"yps{qc}{p}{h}")
               for h in range(2)]
        pts = {}

        def width(ki):
            return ES - max(0, 128 * ki - ES * qc)

        def emit_pv(ki):
            w = width(ki)
            pt = pts.pop(ki)
            for h in range(2):
                head = 2 * p + h
                nc.tensor.matmul(
                    yps[h][:, ES - w:ES],
                    vsb[ki][:, head * 65:head * 65 + 65],
                    pt[:, h * w:h * w + w],
                    start=(ki == 0), stop=(ki == nki - 1))

        for ki in range(nki):
            w = width(ki)
            qs = max(ES * qc, 128 * ki)
            s = spool.tile([128, 1024], F32, tag="S", name="satt")
            for h in range(2):
                nc.tensor.matmul(
                    s[:, 512 * h:512 * h + w],
                    kt[64 * h:64 * h + 64, ki * 128:(ki + 1) * 128],
                    qt[64 * h:64 * h + 64, qs:qs + w],
                    start=True, stop=True, tile_position=(64 * h, 0))
            pt = ppool.tile([128, 1024], BF16, tag="P", name="pt")
            pts[ki] = pt
            nc.scalar.activation(
                pt[:, 0:2 * w].rearrange("p (b x) -> p b x", b=2),
                s[:].rearrange("p (b x) -> p b x", b=2)[:, :, 0:w],
                AF.Exp, scale=0.125)
            if 128 * ki >= ES * qc:
                for h in range(2):
                    nc.vector.tensor_mul(
                        pt[:, h * w:h * w + 128], pt[:, h * w:h * w + 128],
                        mask[:])
            if ki >= 2:
                emit_pv(ki - 2)
        emit_pv(nki - 2)
        emit_pv(nki - 1)
        for h in range(2):
            srow = sbuf2.tile([1, ES], F32, tag="srow", name="srow")
            nc.vector.tensor_copy(srow[:], yps[h][64:65, :])
            rec = sbuf2.tile([1, ES], F32, tag="rec", name="rec")
            nc.vector.reciprocal_approx_fast(rec[:], srow[:])
            bc = sbuf2.tile([64, ES], F32, tag="bc", name="bc")
            nc.gpsimd.partition_broadcast(bc[:], rec[:])
            nc.vector.tensor_mul(
                yn[p][64 * h:64 * h + 64, qc * ES:(qc + 1) * ES],
                yps[h][0:64, :], bc[:])
            # ship each half as soon as its normalize lands (earlier AG)
            nc.sync.dma_start(
                d["ynd"][256 * qc + 128 * p + 64 * h:
                         256 * qc + 128 * p + 64 * h + 64, :],
                yn[p][64 * h:64 * h + 64, qc * ES:(qc + 1) * ES])

    # ---- output stage: per-pair AllGather + c_out-sharded projection -------
    # ynd rows 256qc+128p..+128 = this core's y^T for (qc, pair p).
    # ag rows 512*(2qc+p) + 128*r..+128 = core r's block for (qc, p).
    def emit_ag(qc, p):
        half = 2 * qc + p
        nc.gpsimd.collective_compute(
            "AllGather",
            mybir.AluOpType.bypass,
            replica_groups=GROUPS,
            ins=[d["ynd"][256 * qc + 128 * p:256 * qc + 128 * p + 128,
                          :].opt()],
            outs=[d["ag"][512 * half:512 * (half + 1), :].opt()],
        )

    def emit_proj(qc, tail=False):
        # yt col-block k = y^T rows [128k:128k+128] of this q-chunk; block k
        # lives in AG half (qc, p=k%2) at core r=k//2's slot.
        yt = sbuf2.tile([128, 8 * ES], BF16, tag="yt", name=f"yt{qc}")
        for k in range(NCT):
            r, p = k // 2, k % 2
            row = 512 * (2 * qc + p) + 128 * r
            eng = (nc.sync if k % 2 == 0 else nc.scalar) if tail else nc.sync
            eng.dma_start(
                yt[:, k * ES:(k + 1) * ES],
                d["ag"][row:row + 128, :])
        # out^T[cc*128:+128, chunk] = sum_k wp[k][:,cc].T @ yT[k]; k-outer so
        # the matmuls pace with the yt arrivals.
        pout = sbuf2.tile([128, 2 * ES], F16, tag="pout", name=f"pout{qc}")
        ps = spool.tile([128, 1024], F32, tag="S", name=f"projps{qc}")
        for k in range(NCT):
            for cc in range(2):
                nc.tensor.matmul(
                    ps[:, cc * 512:cc * 512 + 512],
                    wp[:, k, cc * 128:(cc + 1) * 128],
                    yt[:, k * ES:(k + 1) * ES],
                    start=(k == 0), stop=(k == NCT - 1))
        for cc in range(2):
            nc.vector.tensor_scalar_add(
                pout[:, cc * ES:(cc + 1) * ES],
                ps[:, cc * 512:cc * 512 + 512], bpt[:, cc:cc + 1])
            nc.sync.dma_start(
                d["out"][cc * 128:(cc + 1) * 128, qc * ES:(qc + 1) * ES],
                pout[:, cc * ES:(cc + 1) * ES])

    # ---- schedule ----------------------------------------------------------
    # Ascending chunks; AG(qc,p) fires right after pair p's y^T store;
    # proj(qc) lands ~two chunks later so PE never head-of-line blocks on a
    # collective; proj(2) fills the final AG(3,1) wait; only AG(3,1)+proj(3)
    # are exposed.
    emit_qk_quarter(0)
    emit_v_quarter(0)
    emit_att(0, 0)
    emit_ag(0, 0)
    emit_att(0, 1)
    emit_ag(0, 1)
    emit_qk_quarter(1)
    emit_v_quarter(1)
    emit_att(1, 0)
    emit_ag(1, 0)
    emit_att(1, 1)
    emit_ag(1, 1)
    emit_qk_quarter(2)
    emit_v_quarter(2)
    emit_att(2, 0)
    emit_ag(2, 0)
    emit_att(2, 1)
    emit_ag(2, 1)
    emit_proj(0)
    emit_qk_quarter(3)
    emit_v_quarter(3)
    emit_att(3, 0)
    emit_ag(3, 0)
    emit_att(3, 1)
    emit_ag(3, 1)
    emit_proj(1)
    emit_proj(2)
    emit_proj(3, tail=True)


def build_nc():
    global _CACHED_NC
    if _CACHED_NC is not None:
        return _CACHED_NC
    nc = bacc.Bacc("TRN2", target_bir_lowering=False, debug=False,
                   num_devices=N_CORES)
    d = {
        # all host-pre-tiled to [128, ...] partition-major
        "xT": nc.dram_tensor("xT", [128, 4 * NCT * 512], BF16,
                             kind="ExternalInput").ap(),
        "w_qk": nc.dram_tensor("w_qk", [128, 2 * NCT * 256], BF16,
                               kind="ExternalInput").ap(),
        "b_qk": nc.dram_tensor("b_qk", [128, 4], F32,
                               kind="ExternalInput").ap(),
        "w_v": nc.dram_tensor("w_v", [128, NCT * DLOC], BF16,
                              kind="ExternalInput").ap(),
        "b_v": nc.dram_tensor("b_v", [1, DLOC], BF16,
                              kind="ExternalInput").ap(),
        "w_proj": nc.dram_tensor("w_proj", [128, NCT * DLOC], BF16,
                                 kind="ExternalInput").ap(),
        "b_p": nc.dram_tensor("b_p", [128, 2], F32,
                              kind="ExternalInput").ap(),
        "mask": nc.dram_tensor("mask", [128, 128], BF16,
                               kind="ExternalInput").ap(),
        "out": nc.dram_tensor("out", [DLOC, T], F16,
                              kind="ExternalOutput").ap(),
        "ynd": nc.dram_tensor("ynd", [4 * DLOC, ES], BF16).ap(),
        "ag": nc.dram_tensor("ag", [4 * C, ES], BF16).ap(),
        "warmag": nc.dram_tensor("warmag", [5, 64], BF16).ap(),
    }
    from contextlib import ExitStack
    with tile.TileContext(nc) as tc, ExitStack() as es:
        build_kernel_body(nc, tc, es, d)
    nc.compile()
    _CACHED_NC = nc
    return nc


def make_in_maps(x, w_attn, b_attn, w_proj, b_proj):
    x = np.asarray(x, dtype=np.float32)
    w_attn = np.asarray(w_attn, dtype=np.float32)
    b_attn = np.asarray(b_attn, dtype=np.float32)
    w_proj = np.asarray(w_proj, dtype=np.float32)
    b_proj = np.asarray(b_proj, dtype=np.float32)

    # causal mask for the S^T-layout diagonal block: valid iff q >= k
    kr = np.arange(128)
    mask = (kr[None, :] >= kr[:, None]).astype(BF16_NP)  # [k,q]

    def ctile(a):
        """[1024, M] -> [128, 8*M]: partition p, block c = row 128c+p."""
        m = a.shape[1]
        return np.ascontiguousarray(
            a.reshape(NCT, 128, m).transpose(1, 0, 2).reshape(128, NCT * m))

    in_maps = []
    for i in range(N_CORES):
        b = i // TP
        g = i % TP
        heads = list(range(HLOC * g, HLOC * g + HLOC))
        qcols = np.concatenate(
            [np.arange(h * HD, (h + 1) * HD) for h in heads])
        kcols = qcols + C
        vcols = qcols + 2 * C
        # w_qk halves: [m01 = q01|k01 c-tiled] then [m23 = q23|k23 c-tiled]
        q_w = w_attn[:, qcols]
        k_w = w_attn[:, kcols]
        m01 = np.concatenate([q_w[:, 0:128], k_w[:, 0:128]], axis=1)
        m23 = np.concatenate([q_w[:, 128:256], k_w[:, 128:256]], axis=1)
        w_qk = np.concatenate([ctile(m01), ctile(m23)], axis=1)
        b_q = b_attn[qcols]
        b_k = b_attn[kcols]
        b_qk = np.stack([b_q[0:128], b_k[0:128], b_q[128:256],
                         b_k[128:256]], axis=1)           # [128, 4]
        xT = x[b].T  # [1024, 2048]
        # [p, tq, c, m] layout, flattened
        xTt = np.ascontiguousarray(
            xT.reshape(NCT, 128, 4, 512).transpose(1, 2, 0, 3)
            .reshape(128, 4 * NCT * 512))
        in_maps.append({
            "xT": xTt.astype(BF16_NP),
            "w_qk": w_qk.astype(BF16_NP),
            "b_qk": b_qk.astype(np.float32),
            "w_v": ctile(w_attn[:, vcols]).astype(BF16_NP),
            "b_v": b_attn[vcols][None, :].astype(BF16_NP),
            "w_proj": ctile(w_proj[:, DLOC * g:DLOC * (g + 1)]).astype(
                BF16_NP),
            "b_p": np.stack([b_proj[DLOC * g:DLOC * g + 128],
                             b_proj[DLOC * g + 128:DLOC * g + 256]],
                            axis=1).astype(np.float32),   # [128, 2]
            "mask": mask,
        })
    return in_maps


def run(x, w_attn, b_attn, w_proj, b_proj, trace=False):
    _install_ntff_hook()
    nc = build_nc()
    in_maps = make_in_maps(x, w_attn, b_attn, w_proj, b_proj)
    res = run_bass_kernel_spmd(nc, in_maps, list(range(N_CORES)), trace=trace)
    out = np.empty((B, T, C), dtype=np.float32)
    for i in range(N_CORES):
        b = i // TP
        g = i % TP
        o = res.results[i]["out"].astype(np.float32)  # [256 c, 2048 t]
        out[b, :, DLOC * g:DLOC * (g + 1)] = o.T
    return out, res


def kernel(x, w_attn, b_attn, w_proj, b_proj):
    out, _ = run(x, w_attn, b_attn, w_proj, b_proj, trace=False)
    return out
